# revision 12
# baseline (speedup 1.0000x reference)
"""Trainium2 Bass kernel for nn_EncodingLayer_47261820125416.

ALBERT-style encoder layer integrated with 4 fixed RKF56 steps (24 dyn()
evaluations). B=4 batch elements are data-parallel: core c computes batch
c % 4 entirely on-chip (attention + FFN mix all tokens/dims of a batch
element, and collective latency >> any possible gain, so no cross-core
communication is used; cores 4-7 run duplicate batches).

Per-core design (S=512 tokens, d=64, H=8 heads of dh=8, DFF=256):
- State kept transposed on-chip: yT [64, S] fp32, with a ones-row appended so
  every projection bias rides along as an extra contraction row (K=65).
- Scores are computed TRANSPOSED (scoresT[h] = [keys, queries]) with 16-way
  32x32 PE tiling: a padded/duplicated head layout (each 32-partition group
  holds one head's 8 dims) lets one 16-tile pack compute 2 heads x 2
  key-chunks x 512 queries in ~330ns.
- softmax: exp on ACT (the only exp engine; it is the global bottleneck at
  ~16us/dyn) reading [128, 1024] PSUM spans; no max subtraction (scores are
  bounded ~22, validated numerically); the additive mask is a no-op under
  softmax (it broadcasts over keys) and is ignored.
- The softmax denominator Z comes out of the attn@V matmul for free via a
  ones-column appended to V; AV consumes exp-scores as bf16 FWL stationaries
  ([128k, 128q] blocks) producing ctx in [queries, head-major] layout where
  1/Z is a per-partition scalar.
- FFN/projections in bf16, RK linear combinations via fp32r matmuls against
  precomputed coefficient matrices; y accumulated in fp32 on DVE.
"""

import numpy as np
import ml_dtypes

DEPTH, HEADS, DFF = 64, 8, 256
DH = DEPTH // HEADS
S = 512
B = 4
N_CORES = 8
T_FINAL, DT = 1.0, 0.25
N_STEPS = int(round(T_FINAL / DT))
SCALE = 1.0 / float(np.sqrt(DH))

# RKF56 (Fehlberg 6-stage) coefficients
A_COEF = [
    [],
    [0.25],
    [3.0 / 32.0, 9.0 / 32.0],
    [1932.0 / 2197.0, -7200.0 / 2197.0, 7296.0 / 2197.0],
    [439.0 / 216.0, -8.0, 3680.0 / 513.0, -845.0 / 4104.0],
    [-8.0 / 27.0, 2.0, -3544.0 / 2565.0, 1859.0 / 4104.0, -11.0 / 40.0],
]
B_COEF = [16.0 / 135.0, 0.0, 6656.0 / 12825.0, 28561.0 / 56430.0, -9.0 / 50.0, 2.0 / 55.0]


def _coef_list():
    """[(stage, [(pair_idx, c_even, c_odd), ...])] for stages 1..5 and final."""
    out = []
    for i in range(1, 6):
        a = A_COEF[i]
        pairs = []
        for pi in range((len(a) + 1) // 2):
            ce = a[2 * pi]
            co = a[2 * pi + 1] if 2 * pi + 1 < len(a) else 0.0
            pairs.append((pi, ce, co))
        out.append(pairs)
    fb = []
    for pi in range(3):
        fb.append((pi, B_COEF[2 * pi], B_COEF[2 * pi + 1]))
    out.append(fb)
    return out


def prepare_aux(inputs):
    """Host-side numpy preparation of padded/duplicated weight layouts."""
    f32 = np.float32
    bf16 = ml_dtypes.bfloat16
    Wq, bq = np.asarray(inputs["Wq"], f32), np.asarray(inputs["bq"], f32)
    Wk, bk = np.asarray(inputs["Wk"], f32), np.asarray(inputs["bk"], f32)
    Wv, bv = np.asarray(inputs["Wv"], f32), np.asarray(inputs["bv"], f32)
    Wo, bo = np.asarray(inputs["Wo"], f32), np.asarray(inputs["bo"], f32)
    W1, b1 = np.asarray(inputs["W1"], f32), np.asarray(inputs["b1"], f32)
    W2, b2 = np.asarray(inputs["W2"], f32), np.asarray(inputs["b2"], f32)

    # wqk[w, hp, 65, 128]: for head pair hp = (2hp, 2hp+1), column layout
    # [headA pad32 | headB pad32 | headA pad32 | headB pad32], row 64 = bias.
    wqk = np.zeros((2, 4, DEPTH + 1, 128), f32)
    for wi, (W, bias) in enumerate(((Wq, bq), (Wk, bk))):
        for hp in range(4):
            for r in range(4):
                h = 2 * hp + (r % 2)
                wqk[wi, hp, :DEPTH, 32 * r:32 * r + DH] = W[:, h * DH:(h + 1) * DH]
                wqk[wi, hp, DEPTH, 32 * r:32 * r + DH] = bias[h * DH:(h + 1) * DH]

    wv_aug = np.concatenate([Wv, bv[None, :]], 0).astype(f32)          # [65, 64]
    wo_aug = np.concatenate([Wo, bo[None, :]], 0).astype(bf16)         # [65, 64]
    w1_aug = np.concatenate([W1, b1[None, :]], 0).astype(bf16)         # [65, 256]
    w2_ch = W2.reshape(2, 128, DFF // 256 * DEPTH).astype(bf16)        # [2, 128, 64]

    coefs = []
    eye = np.eye(DEPTH, dtype=f32)
    for pairs in _coef_list():
        for (pi, ce, co) in pairs:
            m = np.zeros((128, DEPTH), f32)
            m[:DEPTH] = ce * eye
            m[DEPTH:] = co * eye
            coefs.append(m)
    coefs = np.stack(coefs)                                            # [12, 128, 64]

    ident = np.eye(128, dtype=f32)
    identb = np.eye(128, dtype=bf16)

    return {
        "wqk": wqk,
        "wv_aug": wv_aug,
        "wo_aug": wo_aug,
        "w1_aug": w1_aug,
        "w2_ch": np.ascontiguousarray(w2_ch),
        "b2_col": np.ascontiguousarray(b2.reshape(DEPTH, 1)),
        "coefs": coefs,
        "ident": ident,
        "identb": identb,
    }


def build_module(n_steps=N_STEPS):
    import concourse.bass as bass  # noqa: F401
    import concourse.mybir as mybir
    import concourse.tile as tile
    from concourse import bacc

    f32 = mybir.dt.float32
    f32r = mybir.dt.float32r
    bf = mybir.dt.bfloat16

    nc = bacc.Bacc("TRN2", target_bir_lowering=False, debug=False, enable_asserts=False)

    # --- DRAM I/O ---
    x_d = nc.dram_tensor("x", [S, DEPTH], f32, kind="ExternalInput").ap()
    wqk_d = nc.dram_tensor("wqk", [2, 4, DEPTH + 1, 128], f32r, kind="ExternalInput").ap()
    wv_d = nc.dram_tensor("wv_aug", [DEPTH + 1, DEPTH], f32r, kind="ExternalInput").ap()
    wo_d = nc.dram_tensor("wo_aug", [DEPTH + 1, DEPTH], bf, kind="ExternalInput").ap()
    w1_d = nc.dram_tensor("w1_aug", [DEPTH + 1, DFF], bf, kind="ExternalInput").ap()
    w2_d = nc.dram_tensor("w2_ch", [2, 128, DEPTH], bf, kind="ExternalInput").ap()
    b2_d = nc.dram_tensor("b2_col", [DEPTH, 1], f32, kind="ExternalInput").ap()
    coef_d = nc.dram_tensor("coefs", [12, 128, DEPTH], f32r, kind="ExternalInput").ap()
    id_d = nc.dram_tensor("ident", [128, 128], f32, kind="ExternalInput").ap()
    idb_d = nc.dram_tensor("identb", [128, 128], bf, kind="ExternalInput").ap()
    y_d = nc.dram_tensor("y_out", [S, DEPTH], f32, kind="ExternalOutput").ap()

    coef_stage_index = []
    idx = 0
    for pairs in _coef_list():
        entry = []
        for (pi, _, _) in pairs:
            entry.append((pi, idx))
            idx += 1
        coef_stage_index.append(entry)

    with tile.TileContext(nc) as tc:
        with (
            tc.tile_pool(name="persist", bufs=1) as pp,
            tc.tile_pool(name="psc", bufs=2, space="PSUM") as psc,
            tc.tile_pool(name="pctx", bufs=1, space="PSUM") as pctx,
            tc.tile_pool(name="psm", bufs=2, space="PSUM") as psm,
            tc.tile_pool(name="ph", bufs=1, space="PSUM") as ph,
        ):
            # ---- persistent SBUF ----
            ident = pp.tile([128, 128], f32)
            identb = pp.tile([128, 128], bf)
            wqk_sb = pp.tile([DEPTH + 1, 8, 128], f32r)     # [p, (w hp), m]
            wv_sb = pp.tile([DEPTH + 1, DEPTH], f32r)
            wo_sb = pp.tile([DEPTH + 1, DEPTH], bf)
            w1_sb = pp.tile([DEPTH + 1, DFF], bf)
            w2_sb = pp.tile([128, 2, DEPTH], bf)
            b2_sb = pp.tile([DEPTH, 1], f32)
            coef_sb = pp.tile([128, 12, DEPTH], f32r)
            yT = pp.tile([DEPTH + 1, S], f32)
            ypT = pp.tile([DEPTH + 1, 2, S], f32r)
            kp_sb = pp.tile([128, 3, S], f32r)
            qt_sb = pp.tile([128, 4, S], bf)
            kt_sb = pp.tile([128, 4, S], bf)
            v_sb = pp.tile([128, 4, 72], bf)
            expT = pp.tile([128, HEADS, 2, 1024], bf)
            ctxn = pp.tile([128, 4, DEPTH], bf)
            ctxT_sb = pp.tile([DEPTH + 1, S], bf)
            zT_sb = pp.tile([DEPTH + 1, S], bf)
            hT_sb = pp.tile([128, 2, S], bf)
            recipZ = pp.tile([128, 4, HEADS], f32)
            xn_sb = pp.tile([128, 4, DEPTH], f32)
            yout_sb = pp.tile([128, 4, DEPTH], f32)

            # ---- prologue: weight DMAs + presets ----
            nc.sync.dma_start(ident, id_d)
            nc.sync.dma_start(identb, idb_d)
            nc.sync.dma_start(wqk_sb, wqk_d.rearrange("w h p m -> p (w h) m"))
            nc.sync.dma_start(wv_sb, wv_d)
            nc.sync.dma_start(wo_sb, wo_d)
            nc.sync.dma_start(w1_sb, w1_d)
            nc.sync.dma_start(w2_sb, w2_d.rearrange("c p m -> p c m"))
            nc.sync.dma_start(b2_sb, b2_d)
            nc.sync.dma_start(coef_sb, coef_d.rearrange("s p m -> p s m"))
            nc.sync.dma_start(xn_sb, x_d.rearrange("(c p) d -> p c d", p=128))

            nc.vector.memset(kp_sb.bitcast(f32), 0.0)
            nc.vector.memset(yT[DEPTH:DEPTH + 1, :], 1.0)
            nc.vector.memset(ypT.bitcast(f32)[DEPTH:DEPTH + 1, :, :], 1.0)
            nc.vector.memset(ctxT_sb[DEPTH:DEPTH + 1, :], 1.0)
            nc.vector.memset(zT_sb[DEPTH:DEPTH + 1, :], 1.0)
            v4 = v_sb.rearrange("p c (h n) -> p c h n", h=HEADS)
            nc.vector.memset(v_sb, 1.0)  # ones column at [..., 8] survives V writes

            tc.strict_bb_all_engine_barrier()

            # transpose input x -> yT
            yt0 = psm.tile([DEPTH, S], f32, tag="sm")
            for qc in range(4):
                nc.tensor.transpose(
                    yt0[:, 128 * qc:128 * qc + 128], xn_sb[:, qc, :], ident
                )
            nc.vector.tensor_copy(yT[:DEPTH, :], yt0)

            def emit_dyn(yp_ap, k_slot):
                """One dyn() evaluation reading fp32 [65, S] yp_ap, writing
                DT*k into kp_sb slot k_slot (0..5)."""
                ypr = yp_ap

                # --- QKV projections ---
                for wave in range(2):
                    tq = psc.tile([128, 1024], f32, tag="sc")
                    tk = psc.tile([128, 1024], f32, tag="sc")
                    for i in range(2):
                        hp = 2 * wave + i
                        nc.tensor.matmul(
                            tq[:, 512 * i:512 * i + 512],
                            wqk_sb[:, hp, :], ypr,
                        )
                        nc.tensor.matmul(
                            tk[:, 512 * i:512 * i + 512],
                            wqk_sb[:, 4 + hp, :], ypr,
                        )
                    for i in range(2):
                        hp = 2 * wave + i
                        nc.vector.tensor_copy(qt_sb[:, hp, :], tq[:, 512 * i:512 * i + 512])
                        nc.vector.tensor_copy(kt_sb[:, hp, :], tk[:, 512 * i:512 * i + 512])
                tv = ph.tile([128, S], f32, tag="h")
                for c in range(4):
                    nc.tensor.matmul(
                        tv[:, DEPTH * c:DEPTH * (c + 1)],
                        ypr[:, 128 * c:128 * c + 128], wv_sb,
                    )
                nc.vector.tensor_copy(
                    v4[:, :, :, :DH],
                    tv[:, :4 * DEPTH].rearrange("p (c h n) -> p c h n", c=4, h=HEADS),
                )

                # --- scores + exp + AV, interleaved per head-pair ---
                def emit_pack(hp, kp):
                    tiles = []
                    for hsel in range(2):
                        t = psc.tile([128, 1024], f32, tag="sc")
                        tiles.append(t)
                        for rr in range(2):
                            r = hsel + 2 * rr  # row group; rr selects key chunk
                            kc = 2 * kp + rr
                            for c in range(4):
                                nc.tensor.matmul(
                                    t[32 * c:32 * c + 32, 512 * rr:512 * rr + 512],
                                    kt_sb[:, hp, :][
                                        32 * r:32 * r + 32,
                                        128 * kc + 32 * c:128 * kc + 32 * c + 32,
                                    ],
                                    qt_sb[:, hp, :][32 * r:32 * r + 32, :],
                                    tile_position=(32 * r, 32 * c),
                                )
                    for hsel in range(2):
                        h = 2 * hp + hsel
                        nc.scalar.activation(
                            expT[:, h, kp, :], tiles[hsel][:, :],
                            mybir.ActivationFunctionType.Exp, scale=SCALE,
                        )

                def emit_av(h):
                    for qc in range(4):
                        for kc in range(4):
                            nc.tensor.matmul(
                                ctx_ps[:, 72 * qc + 9 * h:72 * qc + 9 * h + 9],
                                expT[:, h, kc // 2,
                                     (kc % 2) * 512 + 128 * qc:(kc % 2) * 512 + 128 * qc + 128],
                                v_sb[:, kc, 9 * h:9 * h + 9],
                                start=(kc == 0), stop=(kc == 3),
                            )

                ctx_ps = pctx.tile([128, 288], f32, tag="ctx")
                emit_pack(0, 0)
                emit_pack(0, 1)
                emit_pack(1, 0)
                emit_pack(1, 1)
                emit_av(0)
                emit_av(1)
                emit_pack(2, 0)
                emit_pack(2, 1)
                emit_av(2)
                emit_av(3)
                emit_pack(3, 0)
                emit_pack(3, 1)
                emit_av(4)
                emit_av(5)
                emit_av(6)
                emit_av(7)

                # --- normalize + transpose ctx ---
                ctxT_ps = psm.tile([DEPTH, S], bf, tag="sm")
                c3 = ctx_ps.rearrange("p (q h n) -> p q h n", q=4, h=HEADS)
                for qc in range(4):
                    nc.vector.reciprocal(recipZ[:, qc, :], c3[:, qc, :, DH])
                    nc.vector.tensor_tensor(
                        ctxn[:, qc, :].rearrange("p (h n) -> p h n", h=HEADS),
                        c3[:, qc, :, :DH],
                        recipZ[:, qc, :, None].to_broadcast((128, HEADS, DH)),
                        mybir.AluOpType.mult,
                    )
                    nc.tensor.transpose(
                        ctxT_ps[:, 128 * qc:128 * qc + 128], ctxn[:, qc, :], identb
                    )
                nc.vector.tensor_copy(ctxT_sb[:DEPTH, :], ctxT_ps)

                # --- output proj + FFN (accumulated into one [64, S] bank) ---
                attk_ps = psm.tile([DEPTH, S], f32, tag="sm")
                nc.tensor.matmul(attk_ps, wo_sb, ctxT_sb, start=True, stop=True)
                nc.vector.tensor_add(zT_sb[:DEPTH, :], yp_ap.bitcast(f32)[:DEPTH, :], attk_ps)
                for ch in range(2):
                    th = ph.tile([128, S], f32, tag="h")
                    nc.tensor.matmul(th, w1_sb[:, 128 * ch:128 * ch + 128], zT_sb)
                    nc.vector.tensor_scalar_max(hT_sb[:, ch, :], th, 0.0)
                for ch in range(2):
                    nc.tensor.matmul(
                        attk_ps, w2_sb[:, ch, :], hT_sb[:, ch, :],
                        start=False, stop=(ch == 1), skip_group_check=True,
                    )
                # k_slot <- (att + ffn + b2) * DT
                nc.vector.tensor_scalar(
                    kp_sb[64 * (k_slot % 2):64 * (k_slot % 2) + 64, k_slot // 2, :],
                    attk_ps, b2_sb, DT,
                    mybir.AluOpType.add, mybir.AluOpType.mult,
                )

            def emit_delta(stage_pairs):
                d_ps = psm.tile([DEPTH, S], f32, tag="sm")
                n = len(stage_pairs)
                for j, (pi, ci) in enumerate(stage_pairs):
                    nc.tensor.matmul(
                        d_ps, coef_sb[:, ci, :],
                        kp_sb[:, pi, :],
                        start=(j == 0), stop=(j == n - 1),
                    )
                return d_ps

            for _ in range(n_steps):
                for st in range(6):
                    if st == 0:
                        yp_ap = ypT[:, 0, :]
                        nc.vector.tensor_copy(yp_ap[:DEPTH, :], yT[:DEPTH, :])
                    else:
                        d_ps = emit_delta(coef_stage_index[st - 1])
                        yp_ap = ypT[:, st % 2, :]
                        nc.vector.tensor_add(yp_ap[:DEPTH, :], yT[:DEPTH, :], d_ps)
                    emit_dyn(yp_ap, st)
                d_ps = emit_delta(coef_stage_index[5])
                nc.vector.tensor_add(yT[:DEPTH, :], yT[:DEPTH, :], d_ps)

            # ---- epilogue: transpose yT back and store ----
            yo = pctx.tile([128, 288], f32, tag="ctx")
            for qc in range(4):
                nc.tensor.transpose(
                    yo[:, DEPTH * qc:DEPTH * (qc + 1)],
                    yT[:DEPTH, 128 * qc:128 * qc + 128],
                    ident[:DEPTH, :DEPTH],
                )
            nc.vector.tensor_copy(
                yout_sb, yo[:, :4 * DEPTH].rearrange("p (c d) -> p c d", c=4)
            )
            nc.sync.dma_start(y_d.rearrange("(c p) d -> p c d", p=128), yout_sb)

    nc.compile()
    return nc


def _run(inputs, **spmd_kwargs):
    x = np.asarray(inputs["x"], np.float32)
    aux = prepare_aux(inputs)

    nc = build_module()

    in_maps = []
    for c in range(N_CORES):
        m = {"x": np.ascontiguousarray(x[c % B])}
        m.update(aux)
        in_maps.append(m)

    from concourse.bass_utils import run_bass_kernel_spmd

    res = run_bass_kernel_spmd(nc, in_maps, core_ids=list(range(N_CORES)), **spmd_kwargs)
    out = np.stack([res.results[c]["y_out"] for c in range(B)]).astype(np.float32)
    return out, res


def kernel(**inputs):
    return _run(inputs)[0]


# revision 28
# speedup vs baseline: 543.9095x; 543.9095x over previous
"""Trainium2 Bass kernel for nn_EncodingLayer_47261820125416.

ALBERT-style encoder layer integrated with 4 fixed RKF56 steps (24 dyn()
evaluations). B=4 batch elements are data-parallel: core c computes batch
c % 4 entirely on-chip (attention + FFN mix all tokens/dims of a batch
element, and collective latency >> any possible gain, so no cross-core
communication is used; cores 4-7 run duplicate batches).

Per-core design (S=512 tokens, d=64, H=8 heads of dh=8, DFF=256):
- State kept transposed on-chip: yT [64, S] fp32, with a ones-row appended so
  every projection bias rides along as an extra contraction row (K=65).
- Scores are computed TRANSPOSED (scoresT[h] = [keys, queries]) with 16-way
  32x32 PE tiling: a padded/duplicated head layout (each 32-partition group
  holds one head's 8 dims) lets one 16-tile pack compute 2 heads x 2
  key-chunks x 512 queries in ~330ns.
- softmax: exp on ACT (the only exp engine; it is the global bottleneck at
  ~16us/dyn) reading [128, 1024] PSUM spans; no max subtraction (scores are
  bounded ~22, validated numerically); the additive mask is a no-op under
  softmax (it broadcasts over keys) and is ignored.
- The softmax denominator Z comes out of the attn@V matmul for free via a
  ones-column appended to V; AV consumes exp-scores as bf16 FWL stationaries
  ([128k, 128q] blocks) producing ctx in [queries, head-major] layout where
  1/Z is a per-partition scalar.
- FFN/projections in bf16, RK linear combinations via fp32r matmuls against
  precomputed coefficient matrices; y accumulated in fp32 on DVE.
"""

import numpy as np
import ml_dtypes

DEPTH, HEADS, DFF = 64, 8, 256
DH = DEPTH // HEADS
S = 512
B = 4
N_CORES = 8
T_FINAL, DT = 1.0, 0.25
N_STEPS = int(round(T_FINAL / DT))
SCALE = 1.0 / float(np.sqrt(DH))

# RKF56 (Fehlberg 6-stage) coefficients
A_COEF = [
    [],
    [0.25],
    [3.0 / 32.0, 9.0 / 32.0],
    [1932.0 / 2197.0, -7200.0 / 2197.0, 7296.0 / 2197.0],
    [439.0 / 216.0, -8.0, 3680.0 / 513.0, -845.0 / 4104.0],
    [-8.0 / 27.0, 2.0, -3544.0 / 2565.0, 1859.0 / 4104.0, -11.0 / 40.0],
]
B_COEF = [16.0 / 135.0, 0.0, 6656.0 / 12825.0, 28561.0 / 56430.0, -9.0 / 50.0, 2.0 / 55.0]


def _partial_coef_list():
    """Coefficient pairs for partial_j = y + sum_{i<=j-2} a_ji k'_i (stages 2..5)
    and partial_final = y + b1 k1 + b3 k3 + b4 k4."""
    out = []
    for i in range(2, 6):
        a = A_COEF[i][:-1]  # exclude the last (pending) k
        pairs = []
        for pi in range((len(a) + 1) // 2):
            ce = a[2 * pi]
            co = a[2 * pi + 1] if 2 * pi + 1 < len(a) else 0.0
            pairs.append((pi, ce, co))
        out.append(pairs)
    out.append([(0, B_COEF[0], 0.0), (1, B_COEF[2], B_COEF[3])])
    return out


def _shortcut_coefs():
    """Per-stage (lo, hi) coefficients of the kp-pair used by the hp0-QK
    shortcut matmul, and which pair it reads. Stage j (1..5) uses k'_{j-1};
    the final/stage-0 shortcut uses (b5 k'5 + b6 k'6) on pair 2."""
    out = []
    for j in range(1, 6):
        a = A_COEF[j][j - 1]
        pair = (j - 1) // 2
        lo, hi = (a, 0.0) if (j - 1) % 2 == 0 else (0.0, a)
        out.append((pair, lo, hi))
    out.append((2, B_COEF[4], B_COEF[5]))
    return out


def _coef_list():
    """[(stage, [(pair_idx, c_even, c_odd), ...])] for stages 1..5 and final."""
    out = []
    for i in range(1, 6):
        a = A_COEF[i]
        pairs = []
        for pi in range((len(a) + 1) // 2):
            ce = a[2 * pi]
            co = a[2 * pi + 1] if 2 * pi + 1 < len(a) else 0.0
            pairs.append((pi, ce, co))
        out.append(pairs)
    fb = []
    for pi in range(3):
        fb.append((pi, B_COEF[2 * pi], B_COEF[2 * pi + 1]))
    out.append(fb)
    return out


def prepare_aux(inputs):
    """Host-side numpy preparation of padded/duplicated weight layouts."""
    f32 = np.float32
    bf16 = ml_dtypes.bfloat16
    Wq, bq = np.asarray(inputs["Wq"], f32), np.asarray(inputs["bq"], f32)
    Wk, bk = np.asarray(inputs["Wk"], f32), np.asarray(inputs["bk"], f32)
    Wv, bv = np.asarray(inputs["Wv"], f32), np.asarray(inputs["bv"], f32)
    Wo, bo = np.asarray(inputs["Wo"], f32), np.asarray(inputs["bo"], f32)
    W1, b1 = np.asarray(inputs["W1"], f32), np.asarray(inputs["b1"], f32)
    W2, b2 = np.asarray(inputs["W2"], f32), np.asarray(inputs["b2"], f32)

    # wqk[w, hp, 65, 128]: for head pair hp = (2hp, 2hp+1), column layout
    # [headA pad32 | headB pad32 | headA pad32 | headB pad32], row 64 = bias.
    wqk = np.zeros((2, 4, DEPTH + 1, 128), f32)
    for wi, (W, bias) in enumerate(((Wq, bq), (Wk, bk))):
        for hp in range(4):
            for r in range(4):
                h = 2 * hp + (r % 2)
                wqk[wi, hp, :DEPTH, 32 * r:32 * r + DH] = W[:, h * DH:(h + 1) * DH]
                wqk[wi, hp, DEPTH, 32 * r:32 * r + DH] = bias[h * DH:(h + 1) * DH]

    wv_aug = np.concatenate([Wv, bv[None, :]], 0).astype(f32)          # [65, 64]
    # bo is folded linearly: z' = y + att_no_bo, relu(z'W1 + (b1 + bo@W1)),
    # and k-evac adds (b2 + bo) per-partition. The proj then has no bias row.
    wo_aug = np.concatenate([Wo, 0 * bo[None, :]], 0).astype(bf16)     # [65, 64]
    w1_aug = np.concatenate([W1, (b1 + bo @ W1)[None, :]], 0).astype(bf16)  # [65, 256]
    w2_ch = W2.reshape(2, 128, DFF // 256 * DEPTH).astype(bf16)        # [2, 128, 64]

    coefs = []
    eye = np.eye(DEPTH, dtype=f32)
    for pairs in _coef_list():
        for (pi, ce, co) in pairs:
            m = np.zeros((128, DEPTH), f32)
            m[:DEPTH] = ce * eye
            m[DEPTH:] = co * eye
            coefs.append(m)
    coefs = np.stack(coefs)                                            # [12, 128, 64]

    pcoefs = []
    eye = np.eye(DEPTH, dtype=f32)
    for pairs in _partial_coef_list():
        for (pi, ce, co) in pairs:
            m = np.zeros((128, DEPTH), f32)
            m[:DEPTH] = ce * eye
            m[DEPTH:] = co * eye
            pcoefs.append(m)
    pcoefs = np.stack(pcoefs)                                          # [8, 128, 64]

    # hp0 shortcut stationaries: [6 stages, 2 (q/k), 128, 128]
    wqk0s = np.zeros((6, 2, 128, 128), f32)
    for si, (pair, lo, hi) in enumerate(_shortcut_coefs()):
        for wi in range(2):
            wqk0s[si, wi, :DEPTH, :] = lo * wqk[wi, 0, :DEPTH, :]
            wqk0s[si, wi, DEPTH:, :] = hi * wqk[wi, 0, :DEPTH, :]

    ident = np.eye(128, dtype=f32)
    identb = np.eye(128, dtype=bf16)

    return {
        "wqk": wqk,
        "wv_aug": wv_aug,
        "wo_aug": wo_aug,
        "w1_aug": w1_aug,
        "w2_ch": np.ascontiguousarray(w2_ch),
        "b2_col": np.ascontiguousarray((b2 + bo).reshape(DEPTH, 1)),
        "coefs": coefs,
        "pcoefs": pcoefs,
        "wqk0s": wqk0s,
        "ident": ident,
        "identb": identb,
    }


def build_module(n_steps=N_STEPS, score_mult=1, ablate="none"):
    import concourse.bass as bass  # noqa: F401
    import concourse.mybir as mybir
    import concourse.tile as tile
    from concourse import bacc

    f32 = mybir.dt.float32
    f32r = mybir.dt.float32r
    bf = mybir.dt.bfloat16

    nc = bacc.Bacc("TRN2", target_bir_lowering=False, debug=False, enable_asserts=False)

    # --- DRAM I/O ---
    x_d = nc.dram_tensor("x", [S, DEPTH], f32, kind="ExternalInput").ap()
    wqk_d = nc.dram_tensor("wqk", [2, 4, DEPTH + 1, 128], f32r, kind="ExternalInput").ap()
    wv_d = nc.dram_tensor("wv_aug", [DEPTH + 1, DEPTH], f32r, kind="ExternalInput").ap()
    wo_d = nc.dram_tensor("wo_aug", [DEPTH + 1, DEPTH], bf, kind="ExternalInput").ap()
    w1_d = nc.dram_tensor("w1_aug", [DEPTH + 1, DFF], bf, kind="ExternalInput").ap()
    w2_d = nc.dram_tensor("w2_ch", [2, 128, DEPTH], bf, kind="ExternalInput").ap()
    b2_d = nc.dram_tensor("b2_col", [DEPTH, 1], f32, kind="ExternalInput").ap()
    coef_d = nc.dram_tensor("coefs", [12, 128, DEPTH], f32r, kind="ExternalInput").ap()
    pcoef_d = nc.dram_tensor("pcoefs", [8, 128, DEPTH], f32r, kind="ExternalInput").ap()
    wqk0s_d = nc.dram_tensor("wqk0s", [6, 2, 128, 128], f32r, kind="ExternalInput").ap()
    id_d = nc.dram_tensor("ident", [128, 128], f32, kind="ExternalInput").ap()
    idb_d = nc.dram_tensor("identb", [128, 128], bf, kind="ExternalInput").ap()
    y_d = nc.dram_tensor("y_out", [S, DEPTH], f32, kind="ExternalOutput").ap()

    coef_stage_index = []
    idx = 0
    for pairs in _coef_list():
        entry = []
        for (pi, _, _) in pairs:
            entry.append((pi, idx))
            idx += 1
        coef_stage_index.append(entry)
    pcoef_stage_index = []
    idx = 0
    for pairs in _partial_coef_list():
        entry = []
        for (pi, _, _) in pairs:
            entry.append((pi, idx))
            idx += 1
        pcoef_stage_index.append(entry)
    shortcut_pairs = _shortcut_coefs()

    with tile.TileContext(nc) as tc:
        with (
            tc.tile_pool(name="persist", bufs=1) as pp,
            tc.tile_pool(name="psc", bufs=2, space="PSUM") as psc,
            tc.tile_pool(name="pctx", bufs=1, space="PSUM") as pctx,
            tc.tile_pool(name="psm", bufs=2, space="PSUM") as psm,
            tc.tile_pool(name="ph", bufs=1, space="PSUM") as ph,
        ):
            # ---- persistent SBUF ----
            ident = pp.tile([128, 128], f32)
            identb = pp.tile([128, 128], bf)
            wqk_sb = pp.tile([DEPTH + 1, 8, 128], f32r)     # [p, (w hp), m]
            wv_sb = pp.tile([DEPTH + 1, DEPTH], f32r)
            wo_sb = pp.tile([DEPTH + 1, DEPTH], bf)
            w1_sb = pp.tile([DEPTH + 1, DFF], bf)
            w2_sb = pp.tile([128, 2, DEPTH], bf)
            b2_sb = pp.tile([DEPTH, 1], f32)
            coef_sb = pp.tile([128, 12, DEPTH], f32r)
            pcoef_sb = pp.tile([128, 8, DEPTH], f32r)
            wqk0s_sb = pp.tile([128, 6, 2, 128], f32r)
            partial_sb = pp.tile([DEPTH + 1, 2, S], f32r)
            yT = pp.tile([DEPTH + 1, S], f32)
            ypT = pp.tile([DEPTH + 1, 2, S], f32r)
            kp_sb = pp.tile([128, 3, S], f32r)
            qkt_sb = pp.tile([128, 4, 2, S], bf)  # [:, hp, 0]=Q, [:, hp, 1]=K
            v_sb = pp.tile([128, 4, 72], bf)
            expT = pp.tile([128, HEADS, 2, 1024], bf)
            ctxn = pp.tile([128, 4, DEPTH], bf)
            ctxT_sb = pp.tile([DEPTH + 1, S], bf)
            zT_sb = pp.tile([DEPTH + 1, S], bf)
            hT_sb = pp.tile([128, 2, S], bf)
            recipZ = pp.tile([128, 4, HEADS], f32)
            xn_sb = pp.tile([128, 4, DEPTH], f32)
            yout_sb = pp.tile([128, 4, DEPTH], f32)

            # ---- prologue: weight DMAs + presets ----
            nc.sync.dma_start(ident, id_d)
            nc.sync.dma_start(identb, idb_d)
            nc.sync.dma_start(wqk_sb, wqk_d.rearrange("w h p m -> p (w h) m"))
            nc.sync.dma_start(wv_sb, wv_d)
            nc.sync.dma_start(wo_sb, wo_d)
            nc.sync.dma_start(w1_sb, w1_d)
            nc.sync.dma_start(w2_sb, w2_d.rearrange("c p m -> p c m"))
            nc.sync.dma_start(b2_sb, b2_d)
            nc.sync.dma_start(coef_sb, coef_d.rearrange("s p m -> p s m"))
            nc.sync.dma_start(pcoef_sb, pcoef_d.rearrange("s p m -> p s m"))
            nc.sync.dma_start(wqk0s_sb, wqk0s_d.rearrange("s w p m -> p s w m"))
            nc.sync.dma_start(xn_sb, x_d.rearrange("(c p) d -> p c d", p=128))

            nc.vector.memset(kp_sb.bitcast(f32), 0.0)
            nc.vector.memset(yT[DEPTH:DEPTH + 1, :], 1.0)
            nc.vector.memset(ypT.bitcast(f32)[DEPTH:DEPTH + 1, :, :], 1.0)
            nc.vector.memset(partial_sb.bitcast(f32)[DEPTH:DEPTH + 1, :, :], 1.0)
            nc.vector.memset(ctxT_sb[DEPTH:DEPTH + 1, :], 1.0)
            nc.vector.memset(zT_sb[DEPTH:DEPTH + 1, :], 1.0)
            v4 = v_sb.rearrange("p c (h n) -> p c h n", h=HEADS)
            nc.vector.memset(v_sb, 1.0)  # ones column at [..., 8] survives V writes
            if ablate == "noexp":
                nc.vector.memset(expT, 1.0)

            tc.strict_bb_all_engine_barrier()

            # transpose input x -> yT
            yt0 = psm.tile([DEPTH, S], f32, tag="sm")
            for qc in range(4):
                nc.tensor.transpose(
                    yt0[:, 128 * qc:128 * qc + 128], xn_sb[:, qc, :], ident
                )
            nc.vector.tensor_copy(yT[:DEPTH, :], yt0)

            def emit_dyn(yp_ap, k_slot, qk0_tile=None, next_sc=None):
                """One dyn() evaluation reading f32r [65, S] yp_ap, writing
                DT*k into kp_sb slot k_slot (0..5).

                qk0_tile: PSUM tile already holding hp0's Q|K (from the
                previous dyn's shortcut). next_sc = (sidx, mm1_rhs, pair_idx,
                pemit, pslot): build the NEXT dyn's hp0 Q|K via
                QK(partial) + scaled-QK(k_latest); returns that tile."""
                ypr = yp_ap

                # --- QKV projections (emitted interleaved with score packs) ---
                def emit_qk(hp):
                    t = psc.tile([128, 1024], f32, tag="sc")
                    nc.tensor.matmul(t[:, 0:512], wqk_sb[:, hp, :], ypr)
                    nc.tensor.matmul(t[:, 512:1024], wqk_sb[:, 4 + hp, :], ypr)
                    nc.vector.tensor_copy(
                        qkt_sb[:, hp, :, :],
                        t.rearrange("p (w n) -> p w n", w=2),
                    )

                def emit_v():
                    tv = ph.tile([128, S], f32, tag="h")
                    for c in range(4):
                        nc.tensor.matmul(
                            tv[:, DEPTH * c:DEPTH * (c + 1)],
                            ypr[:, 128 * c:128 * c + 128], wv_sb,
                        )
                    nc.vector.tensor_copy(
                        v4[:, :, :, :DH],
                        tv[:, :4 * DEPTH].rearrange("p (c h n) -> p c h n", c=4, h=HEADS),
                    )

                # --- scores + exp + AV, interleaved per head-pair ---
                def emit_pack(hp, kp):
                    tiles = []
                    for hsel in range(2):
                        t = psc.tile([128, 1024], f32, tag="sc")
                        tiles.append(t)
                        for rep in range(score_mult):
                            for rr in range(2):
                                r = hsel + 2 * rr  # row group; rr selects key chunk
                                kc = 2 * kp + rr
                                for c in range(4):
                                    nc.tensor.matmul(
                                        t[32 * c:32 * c + 32, 512 * rr:512 * rr + 512],
                                        qkt_sb[:, hp, 1, :][
                                            32 * r:32 * r + 32,
                                            128 * kc + 32 * c:128 * kc + 32 * c + 32,
                                        ],
                                        qkt_sb[:, hp, 0, :][32 * r:32 * r + 32, :],
                                        tile_position=(32 * r, 32 * c),
                                    )
                    if ablate != "noexp":
                        for hsel in range(2):
                            h = 2 * hp + hsel
                            nc.scalar.activation(
                                expT[:, h, kp, :], tiles[hsel][:, :],
                                mybir.ActivationFunctionType.Exp, scale=SCALE,
                            )

                def emit_av(h):
                    for qc in range(4):
                        for kc in range(4):
                            nc.tensor.matmul(
                                ctx_ps[:, 72 * qc + 9 * h:72 * qc + 9 * h + 9],
                                expT[:, h, kc // 2,
                                     (kc % 2) * 512 + 128 * qc:(kc % 2) * 512 + 128 * qc + 128],
                                v_sb[:, kc, 9 * h:9 * h + 9],
                                start=(kc == 0), stop=(kc == 3),
                            )

                if ablate == "notail":
                    for hp in range(4):
                        emit_pack(hp, 0)
                        emit_pack(hp, 1)
                    nc.vector.tensor_scalar(
                        kp_sb[64 * (k_slot % 2):64 * (k_slot % 2) + 64, k_slot // 2, :],
                        yp_ap.bitcast(f32)[:DEPTH, :], b2_sb, DT,
                        mybir.AluOpType.add, mybir.AluOpType.mult,
                    )
                    return
                ctx_ps = pctx.tile([128, 288], f32, tag="ctx")
                ctxT_ps = psm.tile([DEPTH, S], bf, tag="sm")
                attk_ps = psm.tile([DEPTH, S], f32, tag="sm")
                c3 = ctx_ps.rearrange("p (q h n) -> p q h n", q=4, h=HEADS)

                def emit_ctx_half(g):
                    hs = slice(4 * g, 4 * g + 4)
                    for qc in range(4):
                        nc.vector.reciprocal(recipZ[:, qc, hs], c3[:, qc, hs, DH])
                        nc.vector.tensor_tensor(
                            ctxn[:, qc, 32 * g:32 * g + 32].rearrange(
                                "p (h n) -> p h n", h=4),
                            c3[:, qc, hs, :DH],
                            recipZ[:, qc, hs, None].to_broadcast((128, 4, DH)),
                            mybir.AluOpType.mult,
                        )
                        nc.tensor.transpose(
                            ctxT_ps[32 * g:32 * g + 32, 128 * qc:128 * qc + 128],
                            ctxn[:, qc, 32 * g:32 * g + 32], identb,
                            tile_position=(0, 32 * g),
                        )
                    if g == 0:
                        nc.vector.tensor_copy(ctxT_sb[0:32, :], ctxT_ps[0:32, :])
                        nc.tensor.matmul(attk_ps, wo_sb[0:32, :], ctxT_sb[0:32, :],
                                         start=True, stop=False, skip_group_check=True)
                    else:
                        nc.vector.tensor_copy(ctxT_sb[32:DEPTH, :], ctxT_ps[32:DEPTH, :])
                        nc.tensor.matmul(attk_ps, wo_sb[32:DEPTH, :],
                                         ctxT_sb[32:DEPTH, :],
                                         start=False, stop=True, skip_group_check=True)

                if qk0_tile is None:
                    emit_qk(0)
                else:
                    # split evacuation: ACT takes the Q half, DVE the K half
                    nc.scalar.copy(qkt_sb[:, 0, 0, :], qk0_tile[:, 0:512])
                    nc.vector.tensor_copy(qkt_sb[:, 0, 1, :], qk0_tile[:, 512:1024])
                emit_pack(0, 0)
                emit_qk(1)
                emit_pack(0, 1)
                emit_qk(2)
                emit_v()
                emit_pack(1, 0)
                emit_qk(3)
                emit_pack(1, 1)
                emit_av(0)
                emit_pack(2, 0)
                emit_av(1)
                emit_pack(2, 1)
                emit_av(2)
                emit_pack(3, 0)
                emit_av(3)
                emit_ctx_half(0)
                emit_pack(3, 1)
                # next dyn's partial (off the critical path) + shortcut mm1
                qk_next = None
                if next_sc is not None:
                    sidx, mm1_rhs, pair_idx, pemit, pslot = next_sc
                    if pemit is not None:
                        d_p = psm.tile([DEPTH, S], f32, tag="sm")
                        for j, (pi, ci) in enumerate(pemit):
                            nc.tensor.matmul(
                                d_p, pcoef_sb[:, ci, :], kp_sb[:, pi, :],
                                start=(j == 0), stop=(j == len(pemit) - 1),
                            )
                        nc.vector.tensor_add(
                            partial_sb[:DEPTH, pslot, :], yT[:DEPTH, :], d_p)
                        mm1_rhs = partial_sb[:, pslot, :]
                    qk_next = psc.tile([128, 1024], f32, tag="sc")
                    nc.tensor.matmul(qk_next[:, 0:512], wqk_sb[:, 0, :], mm1_rhs,
                                     start=True, stop=False, skip_group_check=True)
                    nc.tensor.matmul(qk_next[:, 512:1024], wqk_sb[:, 4, :], mm1_rhs,
                                     start=True, stop=False, skip_group_check=True)
                emit_av(4)
                emit_av(5)
                emit_av(6)
                emit_av(7)

                emit_ctx_half(1)

                yp_f = yp_ap.bitcast(f32)
                ksl = kp_sb[64 * (k_slot % 2):64 * (k_slot % 2) + 64, k_slot // 2, :]
                if ablate != "noffn":
                    nc.vector.tensor_add(zT_sb[:DEPTH, :], yp_f[:DEPTH, :], attk_ps)
                    for ch in range(2):
                        th = ph.tile([128, S], f32, tag="h")
                        nc.tensor.matmul(th, w1_sb[:, 128 * ch:128 * ch + 128], zT_sb)
                        if ch == 0:
                            nc.scalar.activation(hT_sb[:, ch, :], th,
                                                 mybir.ActivationFunctionType.Relu)
                        else:
                            nc.vector.tensor_scalar_max(hT_sb[:, ch, :], th, 0.0)
                    for ch in range(2):
                        nc.tensor.matmul(
                            attk_ps, w2_sb[:, ch, :], hT_sb[:, ch, :],
                            start=False, stop=(ch == 1), skip_group_check=True,
                        )
                nc.vector.tensor_scalar(
                    ksl, attk_ps, b2_sb, DT,
                    mybir.AluOpType.add, mybir.AluOpType.mult,
                )
                if qk_next is not None:
                    sidx = next_sc[0]
                    pair_idx = next_sc[2]
                    nc.tensor.matmul(qk_next[:, 0:512], wqk0s_sb[:, sidx, 0, :],
                                     kp_sb[:, pair_idx, :],
                                     start=False, stop=True, skip_group_check=True)
                    nc.tensor.matmul(qk_next[:, 512:1024], wqk0s_sb[:, sidx, 1, :],
                                     kp_sb[:, pair_idx, :],
                                     start=False, stop=True, skip_group_check=True)
                return qk_next

            def emit_delta(stage_pairs):
                d_ps = psm.tile([DEPTH, S], f32, tag="sm")
                n = len(stage_pairs)
                for j, (pi, ci) in enumerate(stage_pairs):
                    nc.tensor.matmul(
                        d_ps, coef_sb[:, ci, :],
                        kp_sb[:, pi, :],
                        start=(j == 0), stop=(j == n - 1),
                    )
                return d_ps

            use_sc = (ablate == "none")
            qk0_tile = None
            for step in range(n_steps):
                for st in range(6):
                    if st == 0:
                        yp_ap = ypT[:, 0, :]
                        nc.vector.tensor_copy(yp_ap[:DEPTH, :], yT[:DEPTH, :])
                        if use_sc and qk0_tile is None:
                            # very first dyn: direct hp0 QK from y
                            qk0_tile = psc.tile([128, 1024], f32, tag="sc")
                            nc.tensor.matmul(qk0_tile[:, 0:512], wqk_sb[:, 0, :], yp_ap)
                            nc.tensor.matmul(qk0_tile[:, 512:1024], wqk_sb[:, 4, :], yp_ap)
                    else:
                        d_ps = emit_delta(coef_stage_index[st - 1])
                        yp_ap = ypT[:, st % 2, :]
                        nc.vector.tensor_add(yp_ap[:DEPTH, :], yT[:DEPTH, :], d_ps)
                    if use_sc:
                        # shortcut descriptor for the NEXT dyn's hp0 QK
                        last = (step == n_steps - 1) and (st == 5)
                        if last:
                            next_sc = None
                        elif st < 5:
                            j = st + 1  # next stage 1..5, shortcut idx j-1
                            pemit = (None if j < 2 else
                                     pcoef_stage_index[j - 2])
                            next_sc = (j - 1,
                                       ypT[:, 0, :] if j == 1 else None,
                                       (j - 1) // 2, pemit, j % 2)
                        else:
                            # next is stage 0 of the next step: partial_f route
                            next_sc = (5, None, 2, pcoef_stage_index[4], 0)
                        qk0_tile = emit_dyn(yp_ap, st, qk0_tile, next_sc)
                    else:
                        emit_dyn(yp_ap, st)
                d_ps = emit_delta(coef_stage_index[5])
                nc.vector.tensor_add(yT[:DEPTH, :], yT[:DEPTH, :], d_ps)

            # ---- epilogue: transpose yT back and store ----
            yo = pctx.tile([128, 288], f32, tag="ctx")
            for qc in range(4):
                nc.tensor.transpose(
                    yo[:, DEPTH * qc:DEPTH * (qc + 1)],
                    yT[:DEPTH, 128 * qc:128 * qc + 128],
                    ident[:DEPTH, :DEPTH],
                )
            nc.vector.tensor_copy(
                yout_sb, yo[:, :4 * DEPTH].rearrange("p (c d) -> p c d", c=4)
            )
            nc.sync.dma_start(y_d.rearrange("(c p) d -> p c d", p=128), yout_sb)

    nc.compile()
    return nc


def _run(inputs, **spmd_kwargs):
    x = np.asarray(inputs["x"], np.float32)
    aux = prepare_aux(inputs)

    nc = build_module()

    in_maps = []
    for c in range(N_CORES):
        m = {"x": np.ascontiguousarray(x[c % B])}
        m.update(aux)
        in_maps.append(m)

    from concourse.bass_utils import run_bass_kernel_spmd

    res = run_bass_kernel_spmd(nc, in_maps, core_ids=list(range(N_CORES)), **spmd_kwargs)
    out = np.stack([res.results[c]["y_out"] for c in range(B)]).astype(np.float32)
    return out, res


def kernel(**inputs):
    return _run(inputs)[0]
